# revision 41
# baseline (speedup 1.0000x reference)
"""NT-Xent contrastive loss on 8 Trainium2 NeuronCores (V3, bf16 + XBAR).

Math (reference): z = [z_i; z_j] (N=8192, D=128), zn = z/||z||,
sim = zn@zn.T / 0.1.  Row loss_i = logsumexp_{j!=i} sim[i,j] - sim[i, pos(i)],
loss = mean_i loss_i.

Sharding: rolled-column trick.  Core c receives z rolled by -1024*c rows.
Its 1024 local rows are rolled rows 0..1023; in rolled coordinates the
self column of local row i is i and the positive column is i + 4096 on
EVERY core, so a single static SPMD program works with no collectives.
The self logit is suppressed by adding -5 to the diagonal cosine
(logit -40 -> exp ~4e-18, negligible).  Host sums the 8 partial means.

V3 changes vs V2:
  - znT is bf16, built by XBAR DMA-transpose (dma_start_transpose on the
    SP queue) from bf16-scaled zn tiles: no PE transposes, no PSUM
    contention, no DVE psum->sbuf copies.
  - PSUM holds a true [128,2048] double buffer (8 banks) for the sim
    matmuls, so PE always runs one chunk ahead of ACT's exp stream.
  - Matmuls are bf16 (1 cycle/row; cosine-logit error ~5e-3, loss error
    ~1e-5, far inside the 2e-2 gate).
"""

import os
import sys

import numpy as np

_TRN_REPO = "/opt/trn_rl_repo"
if _TRN_REPO not in sys.path:
    sys.path.insert(0, _TRN_REPO)

from concourse import bacc, bass, mybir, tile
from concourse.bass_utils import run_bass_kernel_spmd

B = 4096
D = 128
N = 2 * B
N_CORES = 8
RPC = N // N_CORES  # 1024 rows per core
INV_T = 10.0
DIAG_SHIFT = -5.0

NBATCH = 4  # stage-A batches of 2048 rows
TPB = 16    # 128-row tiles per batch
RB = 8      # row blocks per core (128 rows each)
QB = 4      # 2048-wide column chunks
KB = 4      # 512-wide matmuls per chunk

_cache: dict = {}


def build():
    f32 = mybir.dt.float32
    bf16 = mybir.dt.bfloat16
    AX = mybir.AxisListType
    AF = mybir.ActivationFunctionType

    nc = bacc.Bacc(
        "TRN2", target_bir_lowering=False, debug=False, num_devices=N_CORES
    )

    # Pin ln/exp/copy/etc to one ACT table (see V2 note): avoids 1.3us
    # ACT_TABLE_LOAD at every ln<->exp transition.
    tabs = bacc.get_activation_tables(nc.m.arch)
    pinned = set(tabs["natural_log_exp_and_others"])
    for k in tabs:
        if k != "natural_log_exp_and_others":
            tabs[k] = tabs[k] - pinned

    z_dram = nc.dram_tensor("z_roll", [N, D], f32, kind="ExternalInput")
    loss_dram = nc.dram_tensor("loss_part", [1, 1], f32, kind="ExternalOutput")

    eye_np = np.eye(128, dtype=np.float32)
    eye_dram = nc.inline_tensor(eye_np, name="eye128")
    negI_dram = nc.inline_tensor(
        (DIAG_SHIFT * eye_np).astype(np.float32), name="negI128"
    )
    ones_dram = nc.inline_tensor(np.ones((128, 1), np.float32), name="ones128")

    with tile.TileContext(nc) as tc:
        with (
            tc.tile_pool(name="const", bufs=1) as cpool,
            tc.tile_pool(name="zin", bufs=NBATCH) as zpool,
            tc.tile_pool(name="zn", bufs=2) as npool,
            tc.tile_pool(name="persist", bufs=1) as ppool,
            tc.tile_pool(name="scr", bufs=2) as spool,
            tc.tile_pool(name="psum", bufs=2, space=bass.MemorySpace.PSUM) as qpool,
        ):
            eye_sb = cpool.tile([128, 128], f32)
            negI_sb = cpool.tile([128, 128], f32)
            ones_sb = cpool.tile([128, 1], f32)

            ssq = ppool.tile([128, NBATCH * TPB], f32)
            lnssq = ppool.tile([128, NBATCH * TPB], f32)
            inv = ppool.tile([128, NBATCH * TPB], f32)
            znT = ppool.tile([128, N], bf16)
            sexp = ppool.tile([128, RB, QB], f32)
            pos = ppool.tile([128, RB], f32)

            # Input DMAs: batch 0 goes alone on the wire first; batches 1-3
            # and the constants are gated behind its arrival via a dummy
            # gpsimd read so they don't steal DMA bandwidth from the
            # critical prologue chain.
            zin_tiles = []
            for b in range(NBATCH):
                zin = zpool.tile([128, TPB, 128], f32)
                zin_tiles.append(zin)

            def zin_dma(b, s, eng=None):
                r0 = 2048 * b + 512 * s
                src = z_dram[r0 : r0 + 512, :].rearrange(
                    "(t p) d -> p t d", p=128
                )
                (eng or nc.gpsimd).dma_start(
                    zin_tiles[b][:, 4 * s : 4 * s + 4, :], src
                )

            # batch 0 spread across all three DMA-capable queues so its four
            # transfers run in parallel and land ~5us sooner
            b0_engs = [nc.sync, nc.scalar, nc.sync, nc.gpsimd]
            for s in range(4):
                zin_dma(0, s, eng=b0_engs[s])
            gate = cpool.tile([128, 4], f32)
            nc.gpsimd.tensor_copy(gate[:], zin_tiles[0][:, 15, 0:4])
            for b in range(1, NBATCH):
                for s in range(4):
                    zin_dma(b, s)
            nc.gpsimd.dma_start(eye_sb[:], eye_dram[:])
            nc.gpsimd.dma_start(negI_sb[:], negI_dram[:])
            nc.gpsimd.dma_start(ones_sb[:], ones_dram[:])

            def ssq_mul(b):
                scr = spool.tile([128, TPB * 128], f32, tag="sq")
                zv = zin_tiles[b][:].rearrange("p t d -> p (t d)")
                nc.vector.tensor_mul(scr[:], zv, zv)
                return scr

            def ssq_red(b, scr):
                j0 = TPB * b
                nc.vector.reduce_sum(
                    ssq[:, j0 : j0 + TPB],
                    scr[:].rearrange("p (t d) -> p t d", d=128),
                    axis=AX.X,
                )

            def norms(b):
                # 1/||z|| = exp(-0.5*ln(ssq)); stays in the Ln/Exp ACT table.
                j0 = TPB * b
                nc.scalar.activation(
                    lnssq[:, j0 : j0 + TPB], ssq[:, j0 : j0 + TPB], AF.Ln
                )
                nc.scalar.activation(
                    inv[:, j0 : j0 + TPB], lnssq[:, j0 : j0 + TPB],
                    AF.Exp, scale=-0.5,
                )

            zn_tiles = {}

            def tsm(b, t0, t1):
                if b not in zn_tiles:
                    zn_tiles[b] = npool.tile(
                        [128, TPB, 128], bf16, name=f"zn{b}", tag="zn"
                    )
                # one broadcast tensor_mul (inv gets a stride-0 last dim)
                # instead of 16 tensor_scalar ops: fewer DVE sems keeps the
                # xbar's inflated cross-engine wait target small.
                zn = zn_tiles[b]
                iv = inv[:, TPB * b + t0 : TPB * b + t1]
                iv_bc = bass.AP(iv.tensor, iv.offset, iv.ap + [[0, 128]])
                nc.vector.tensor_mul(
                    zn[:, t0:t1, :], zin_tiles[b][:, t0:t1, :], iv_bc
                )

            def build_trans(b):
                zn = zn_tiles[b]
                c0 = 2048 * b
                nc.sync.dma_start_transpose(
                    znT[:, c0 : c0 + 2048].rearrange("p (t c) -> p t c", c=128),
                    zn[:].rearrange("p t d -> p (t d)"),
                )

            def ssq_half(b, h):
                scr = spool.tile([128, 8 * 128], f32, tag="sqh")
                zv = zin_tiles[b][:, 8 * h : 8 * h + 8, :].rearrange(
                    "p t d -> p (t d)"
                )
                nc.vector.tensor_mul(scr[:], zv, zv)
                j0 = TPB * b + 8 * h
                nc.vector.reduce_sum(
                    ssq[:, j0 : j0 + 8],
                    scr[:].rearrange("p (t d) -> p t d", d=128),
                    axis=AX.X,
                )

            def norms_half(b, h):
                j0 = TPB * b + 8 * h
                nc.scalar.activation(
                    lnssq[:, j0 : j0 + 8], ssq[:, j0 : j0 + 8], AF.Ln
                )
                nc.scalar.activation(
                    inv[:, j0 : j0 + 8], lnssq[:, j0 : j0 + 8],
                    AF.Exp, scale=-0.5,
                )

            # --- prologue: batch 0 only, ssq/norms/scale pipelined in
            # 1024-row halves under the staggered DMA arrivals ---
            for h in range(2):
                ssq_half(0, h)
                norms_half(0, h)
                tsm(0, 8 * h, 8 * h + 8)
            build_trans(0)

            # --- main loop: q-outer, r-inner; build batch q+1 under chunk q ---
            for q in range(QB):
                b = q + 1
                for r in range(RB):
                    lhsT = znT[:, 128 * r : 128 * (r + 1)]
                    ps = qpool.tile([128, 2048], f32, tag="mm")
                    for k in range(KB):
                        c0 = 2048 * q + 512 * k
                        nc.tensor.matmul(
                            ps[:, 512 * k : 512 * (k + 1)],
                            lhsT,
                            znT[:, c0 : c0 + 512],
                            start=True,
                            stop=True,
                        )
                    if q == 0:
                        sub = ps[:, 128 * r : 128 * (r + 1)]
                        nc.vector.tensor_add(sub, sub, negI_sb[:])
                    if q == 2:
                        scr = spool.tile([128, 128], f32, tag="pos")
                        nc.vector.tensor_mul(
                            scr[:], ps[:, 128 * r : 128 * (r + 1)], eye_sb[:]
                        )
                        nc.vector.reduce_sum(
                            pos[:, r : r + 1], scr[:], axis=AX.X
                        )
                    nc.scalar.activation(
                        ps[:],
                        ps[:],
                        AF.Exp,
                        scale=INV_T,
                        accum_out=sexp[:, r, q : q + 1],
                    )
                    if b < NBATCH:
                        if r == 0:
                            zn_tiles[f"scr{b}"] = ssq_mul(b)
                        elif r == 1:
                            ssq_red(b, zn_tiles[f"scr{b}"])
                        elif r == 2:
                            norms(b)
                        elif r == 3:
                            tsm(b, 0, 8)
                        elif r == 4:
                            tsm(b, 8, TPB)
                        elif r == 5:
                            build_trans(b)

            # --- epilogue ---
            s8 = ppool.tile([128, RB], f32)
            nc.vector.reduce_sum(s8[:], sexp[:], axis=AX.X)
            lse = ppool.tile([128, RB], f32)
            nc.scalar.activation(lse[:], s8[:], AF.Ln)
            poss = ppool.tile([128, RB], f32)
            nc.scalar.mul(poss[:], pos[:], INV_T)
            acc = ppool.tile([128, RB], f32)
            nc.vector.tensor_sub(acc[:], lse[:], poss[:])
            tot = ppool.tile([128, 1], f32)
            nc.vector.reduce_sum(tot[:], acc[:], axis=AX.X)
            psf = qpool.tile([128, 2048], f32, tag="mm")
            nc.tensor.matmul(
                psf[0:1, 0:1], ones_sb[:], tot[:], start=True, stop=True
            )
            res = ppool.tile([1, 1], f32)
            nc.scalar.mul(res[:], psf[0:1, 0:1], 1.0 / N)
            nc.gpsimd.dma_start(loss_dram[:], res[:])

    nc.compile()
    return nc


def get_nc():
    if "nc" not in _cache:
        _cache["nc"] = build()
    return _cache["nc"]


def make_in_maps(z_i: np.ndarray, z_j: np.ndarray):
    z = np.concatenate(
        [np.asarray(z_i, np.float32), np.asarray(z_j, np.float32)], axis=0
    )
    return [
        {"z_roll": np.ascontiguousarray(np.roll(z, -RPC * c, axis=0))}
        for c in range(N_CORES)
    ]


def kernel(**inputs) -> np.ndarray:
    in_maps = make_in_maps(inputs["z_i"], inputs["z_j"])
    nc = get_nc()
    res = run_bass_kernel_spmd(nc, in_maps, list(range(N_CORES)))
    kernel.last_results = res
    total = np.float32(0.0)
    for r in res.results:
        total = np.float32(total + np.float32(np.asarray(r["loss_part"]).reshape(())))
    return np.float32(total)
